# revision 4
# baseline (speedup 1.0000x reference)
"""Multi-head attention (B=2, S=2048, D=1024, H=16) on 8 TRN2 NeuronCores.

Sharding: core c handles batch b = c//4 and head-group g = c%4 (4 heads,
d-slice of 256). Host compacts keys/values by the attention mask (exact:
masked keys contribute exp->0 in the fp32 reference), pads to a multiple
of 128; a valid-flag column excludes padding from numerator/denominator.

Per core (bf16 matmuls throughout, fp32 PSUM accumulation):
  Q^T = WqT.T @ X^T (+bq)          [256, 2048]
  K^T = WkT.T @ Xkv^T (+bk)        [256, SKV]
  V   = Xkv^T-chunks @ WvT         [SKV, 4h, 64+vf]
  per (h, q-block): S^T = K_h Q_h^T, P = exp(S^T/8) on Act (bf16 out)
  psO[65, 512] accumulates [V_h | vf].T @ P over key chunks:
     rows 0..63 = unnormalized O^T, row 64 = softmax denominator
  O^T = psO[0:64] * recip(den)  (DVE recip straight from PSUM,
     GpSimd partition-broadcast)
  OUT_partial = O^T.T @ WoT        [2048, 1024], bf16 out

Scheduling: slots are (q-block, head-PAIR). The two heads of a pair live
in SBUF partitions 0-63 / 64-127 of the same KT/Q column, so their score
matmuls (contraction = d_k = 64) are emitted back-to-back and execute
CONCURRENTLY as 64x128 row-tiles T0/T8 (tile_position auto-derived from
base partitions) - 2x score throughput vs one-head-at-a-time. Between
score groups the slot interleaves ready work (AV of the previous pair,
out-projection chunks, next Q-projection) so the in-order PE queue never
blocks on an exp. A burst of throwaway matmuls on a memset tile warms the
PE HAM clock-gate (1.2->2.4 GHz) during the input-DMA window, and the
K-projection consumes XKV DMA chunks as they land (i-outer loop).
PSUM: scores/exp ring is 2 slots of [128,2,512] (tag s2, 4 banks);
everything else (warmup, V/Q-proj, AV, out-proj) rotates through 4
single-bank b1 slots. Critical inputs (WKT, XKV, biases) ride the SP
descriptor queue; WQT/XT0 and the rest ride the GpSimd queue.

V/O biases fold into a host-side constant: A@(V+bv)Wo^T + bo =
A@V@Wo^T + (bv@Wo^T + bo). Partial outputs over head-groups are summed
on the host.
"""

import math
import os
from functools import lru_cache

import numpy as np

D_MODEL = 1024
NUM_HEADS = 16
D_K = 64


class _ActCopy:
    """tensor_copy shim routing through the Act engine's activation-Copy."""

    def __init__(self, nc):
        self.nc = nc

    def tensor_copy(self, out, in_):
        self.nc.scalar.copy(out, in_)


B = 2
S = 2048
N_CORES = 8
GROUPS = 4          # head-groups = cores per batch
DH = 256            # d-slice per core (4 heads x 64)
NH_LOC = 4          # heads per core
P = 128
CC = D_MODEL // P   # contraction chunks (8)

# results of the last hardware run (BassKernelResults), for test harnesses
last_results = None


@lru_cache(maxsize=2)
def _build(SKV: int):
    import concourse.mybir as mybir
    import concourse.tile as tile
    from concourse import bacc

    f32 = mybir.dt.float32
    bf16 = mybir.dt.bfloat16
    KC = SKV // P
    QB = S // 512                       # q blocks of 512
    NSLOT = QB * 2                      # (q block, head pair) slots
    kc_groups = [list(range(g, min(g + 2, KC))) for g in range(0, KC, 2)]
    kb512 = [(s0, min(512, SKV - s0)) for s0 in range(0, SKV, 512)]

    nc = bacc.Bacc("TRN2", target_bir_lowering=False, debug=False,
                   num_devices=N_CORES)

    XT_d = nc.dram_tensor("xt", [D_MODEL, S], bf16, kind="ExternalInput").ap()
    XKV_d = nc.dram_tensor("xkv", [D_MODEL, SKV], bf16, kind="ExternalInput").ap()
    WQT_d = nc.dram_tensor("wqt", [D_MODEL, DH], bf16, kind="ExternalInput").ap()
    WKT_d = nc.dram_tensor("wkt", [D_MODEL, DH], bf16, kind="ExternalInput").ap()
    WVT_d = nc.dram_tensor("wvt", [D_MODEL, DH], bf16, kind="ExternalInput").ap()
    WOT_d = nc.dram_tensor("wot", [DH, D_MODEL], bf16, kind="ExternalInput").ap()
    bq_d = nc.dram_tensor("bq", [DH], f32, kind="ExternalInput").ap()
    bk_d = nc.dram_tensor("bk", [DH], f32, kind="ExternalInput").ap()
    vf_d = nc.dram_tensor("vf", [SKV], bf16, kind="ExternalInput").ap()
    OUT_d = nc.dram_tensor("out", [S, D_MODEL], bf16, kind="ExternalOutput").ap()

    with tile.TileContext(nc) as tc:
        with tc.tile_pool(name="res", bufs=1) as res, \
             tc.tile_pool(name="ps", bufs=2, space="PSUM") as ps, \
             tc.tile_pool(name="qtp", bufs=4) as qtp, \
             tc.tile_pool(name="ptp", bufs=4) as ptp, \
             tc.tile_pool(name="otp", bufs=4) as otp, \
             tc.tile_pool(name="nrm", bufs=6) as nrm:
            XKV_sb = res.tile([P, CC, SKV], bf16)
            WKT_sb = res.tile([P, CC, DH], bf16)
            WVT_sb = res.tile([P, CC, DH], bf16)
            WQT_sb = res.tile([P, CC, DH], bf16)
            XT_sb = res.tile([P, CC, S], bf16)
            WOT_sb = res.tile([P, 2, D_MODEL], bf16)
            bq_sb = res.tile([P, 2], f32)
            bk_sb = res.tile([P, 2], f32)
            KT_sb = res.tile([P, 2, SKV], bf16)
            V_sb = res.tile([P, KC, NH_LOC, 65], bf16)
            junk = res.tile([P, 640], bf16)

            # ---- PE warm-up: the HAM clock gate keeps the PE at 1.2 GHz
            # until ~3.4us of sustained matmul activity. Burn that window
            # on throwaway matmuls over a memset tile while input DMAs
            # stream, so the real work starts at 2.4 GHz.
            nc.vector.memset(junk[:], 0.0)
            pw = ps.tile([P, 512], f32, tag="b1", name="warm")
            for _ in range(14):
                nc.tensor.matmul(pw[:], junk[:, 0:128], junk[:, 128:640],
                                 start=True, stop=True)

            # ---- input DMAs. SP (sync) queue carries the K-projection
            # critical path in arrival order (WKT, then XKV chunk pairs -
            # the i-outer K-proj below consumes them as they land), then
            # the biases. The GpSimd hardware queue carries everything
            # else, most-urgent first.
            nc.sync.dma_start(WKT_sb[:], WKT_d.rearrange("(c p) d -> p c d", p=P))
            for cq in range(0, CC, 2):
                nc.sync.dma_start(XKV_sb[:, cq:cq + 2, :],
                                  XKV_d.rearrange("(c p) k -> p c k", p=P)
                                      [:, cq:cq + 2, :])
            nc.sync.dma_start(bk_sb[:], bk_d.rearrange("(t p) -> p t", p=P))
            nc.sync.dma_start(bq_sb[:], bq_d.rearrange("(t p) -> p t", p=P))

            nc.gpsimd.dma_start(WQT_sb[:], WQT_d.rearrange("(c p) d -> p c d", p=P))
            nc.gpsimd.dma_start(
                XT_sb[:, :, 0:512],
                XT_d.rearrange("(c p) q -> p c q", p=P)[:, :, 0:512])
            nc.gpsimd.dma_start(WVT_sb[:], WVT_d.rearrange("(c p) d -> p c d", p=P))
            for h in range(NH_LOC):
                nc.gpsimd.dma_start(V_sb[:, :, h, 64],
                                    vf_d.rearrange("(kc p) -> p kc", p=P))
            for qb in range(1, QB):
                nc.gpsimd.dma_start(
                    XT_sb[:, :, qb * 512:(qb + 1) * 512],
                    XT_d.rearrange("(c p) q -> p c q", p=P)
                        [:, :, qb * 512:(qb + 1) * 512])
            nc.gpsimd.dma_start(WOT_sb[:], WOT_d.rearrange("(t p) e -> p t e", p=P))

            def emit_kproj(t):
                # i-outer over contraction chunks: the first matmuls need
                # only XKV chunk 0, so K-proj overlaps the XKV DMA.
                for j0 in range(0, len(kb512), 2):
                    blks = kb512[j0:j0 + 2]
                    if len(blks) == 2:
                        psk = ps.tile([P, 2, 512], f32, tag="s2",
                                      name=f"psk{t}_{j0}")
                        for i in range(CC):
                            for jj, (k0, sz) in enumerate(blks):
                                nc.tensor.matmul(
                                    psk[:, jj, 0:sz],
                                    WKT_sb[:, i, t * P:(t + 1) * P],
                                    XKV_sb[:, i, k0:k0 + sz],
                                    start=(i == 0), stop=(i == CC - 1))
                        lo = blks[0][0]
                        hi = blks[-1][0] + blks[-1][1]
                        nc.vector.tensor_scalar_add(
                            KT_sb[:, t, lo:hi],
                            psk.rearrange("p a b -> p (a b)")[:, 0:hi - lo],
                            bk_sb[:, t:t + 1])
                    else:
                        k0, sz = blks[0]
                        psk = ps.tile([P, 512], f32, tag="b1",
                                      name=f"psk{t}_{j0}")
                        for i in range(CC):
                            nc.tensor.matmul(
                                psk[:, 0:sz],
                                WKT_sb[:, i, t * P:(t + 1) * P],
                                XKV_sb[:, i, k0:k0 + sz],
                                start=(i == 0), stop=(i == CC - 1))
                        nc.vector.tensor_scalar_add(
                            KT_sb[:, t, k0:k0 + sz], psk[:, 0:sz],
                            bk_sb[:, t:t + 1])

            def emit_vproj(kc):
                psv = ps.tile([P, 512], f32, tag="b1", name=f"psv{kc}")
                for i in range(CC):
                    nc.tensor.matmul(
                        psv[:, 0:DH],
                        XKV_sb[:, i, kc * P:(kc + 1) * P],
                        WVT_sb[:, i, :],
                        start=(i == 0), stop=(i == CC - 1))
                nc.vector.tensor_copy(
                    V_sb[:, kc, :, 0:64],
                    psv[:, 0:DH].rearrange("p (h d) -> p h d", h=NH_LOC))

            qts = {}

            def emit_qproj(qb):
                q0 = qb * 512
                qt = qtp.tile([P, 2, 512], bf16, tag="qt", name=f"qt{qb}")
                qts[qb] = qt
                for t in range(2):
                    psq = ps.tile([P, 512], f32, tag="b1", name=f"psq{qb}_{t}")
                    for i in range(CC):
                        nc.tensor.matmul(
                            psq[:],
                            WQT_sb[:, i, t * P:(t + 1) * P],
                            XT_sb[:, i, q0:q0 + 512],
                            start=(i == 0), stop=(i == CC - 1))
                    nc.vector.tensor_scalar_add(
                        qt[:, t, :], psq[:], bq_sb[:, t:t + 1])

            ptts = {}

            def emit_scores(qb, hp, kcs):
                # the two heads of pair hp sit at partitions 0-63 / 64-127
                # of KT/Q column hp; their matmuls alternate per kc and run
                # concurrently as 64x128 row-tiles (0,0)/(64,0).
                qt = qts[qb]
                pts = []
                pss = []
                for eo, po in enumerate((0, 64)):
                    h = 2 * hp + eo
                    if (qb, h) in ptts:
                        ptt = ptts[(qb, h)]
                    else:
                        ptt = ptp.tile([P, KC, 512], bf16, tag="pt",
                                       name=f"pt{qb}_{h}")
                        ptts[(qb, h)] = ptt
                    pts.append(ptt)
                    pss.append(ps.tile([P, 2, 512], f32, tag="s2",
                                       name=f"pss{qb}_{hp}_{eo}_{kcs[0]}"))
                for i, kc in enumerate(kcs):
                    for eo, po in enumerate((0, 64)):
                        nc.tensor.matmul(
                            pss[eo][:, i, :],
                            KT_sb[po:po + 64, hp, kc * P:(kc + 1) * P],
                            qt[po:po + 64, hp, :],
                            start=True, stop=True)
                for eo in range(2):
                    nc.scalar.activation(
                        pts[eo][:, kcs[0]:kcs[0] + len(kcs), :],
                        pss[eo][:, 0:len(kcs), :],
                        mybir.ActivationFunctionType.Exp, scale=0.125)

            ots = {}

            def emit_av(qb, h):
                t, po = h // 2, (h % 2) * 64
                ptt = ptts.pop((qb, h))
                pso = ps.tile([P, 512], f32, tag="b1")
                for kc in range(KC):
                    nc.tensor.matmul(
                        pso[0:65, :],
                        V_sb[:, kc, h, :],
                        ptt[:, kc, :],
                        start=(kc == 0), stop=(kc == KC - 1))
                # NB: reciprocal_approx_fast is a custom DVE op and misreads
                # PSUM sources on HW (sim-only correct) - stage den in SBUF.
                den = nrm.tile([1, 512], f32, tag="den")
                nc.vector.tensor_copy(den[:], pso[64:65, :])
                rec = nrm.tile([1, 512], f32, tag="rec")
                nc.vector.reciprocal_approx_fast(rec[:], den[:])
                recb = nrm.tile([64, 512], f32, tag="recb")
                nc.gpsimd.partition_broadcast(recb[:], rec[:], channels=64)
                if h == 0:
                    ot = otp.tile([P, 2, 512], bf16, tag="ot", name=f"ot{qb}")
                    ots[qb] = ot
                ot = ots[qb]
                nc.vector.tensor_mul(ot[po:po + 64, t, :],
                                     pso[0:64, :], recb[:])

            def emit_oproj_qc(qb, qc, copy_eng=None):
                q0 = qb * 512
                ot = ots[qb]
                ob = nrm.tile([P, 2, 512], bf16, tag="ob")
                if copy_eng is None:
                    copy_eng = nc.vector
                for nb in range(2):
                    pso1 = ps.tile([P, 512], f32, tag="b1",
                                   name=f"op{qb}_{qc}_{nb}")
                    p3 = pso1[:]
                    for t in range(2):
                        nc.tensor.matmul(
                            p3,
                            ot[:, t, qc * P:(qc + 1) * P],
                            WOT_sb[:, t, nb * 512:(nb + 1) * 512],
                            start=(t == 0), stop=(t == 1))
                    copy_eng.tensor_copy(ob[:, nb, :], p3)
                nc.sync.dma_start(
                    OUT_d[q0 + qc * P:q0 + (qc + 1) * P, :], ob[:])

            # ---- software-pipelined emission over (qb, head-pair) slots.
            # Per slot, packed score groups interleave with ready work so
            # the PE stays dense while Act drains the exps.
            emit_kproj(0)
            emit_qproj(0)
            for s in range(NSLOT):
                qb, hp = divmod(s, 2)
                work = []
                if s == 0:
                    vsplit = (KC + 1) // 2
                    work.append(lambda v=vsplit: [emit_vproj(kc)
                                                  for kc in range(0, v)])
                    work.append(lambda v=vsplit: [emit_vproj(kc)
                                                  for kc in range(v, KC)])
                    work.append(lambda: emit_kproj(1))
                    work.append(lambda: emit_qproj(1))
                else:
                    pqb, php = divmod(s - 1, 2)
                    work.append(lambda a=pqb, b=2 * php: emit_av(a, b))
                    work.append(lambda a=pqb, b=2 * php + 1: emit_av(a, b))
                    if qb >= 1:
                        work.append(lambda a=qb - 1, b=2 * hp:
                                    emit_oproj_qc(a, b))
                        work.append(lambda a=qb - 1, b=2 * hp + 1:
                                    emit_oproj_qc(a, b))
                    if hp == 0 and qb + 1 < QB:
                        work.append(lambda a=qb + 1: emit_qproj(a))
                for g, kcs in enumerate(kc_groups):
                    emit_scores(qb, hp, kcs)
                    if g < len(work):
                        work[g]()
                for w in work[len(kc_groups):]:
                    w()
            # tail: last pair's AV, then spread the last block's out-proj
            # over both copy engines (Act is idle by now)
            emit_av(QB - 1, 2)
            emit_av(QB - 1, 3)
            emit_oproj_qc(QB - 1, 0, copy_eng=nc.vector)
            emit_oproj_qc(QB - 1, 1, copy_eng=_ActCopy(nc))
            emit_oproj_qc(QB - 1, 2, copy_eng=_ActCopy(nc))
            emit_oproj_qc(QB - 1, 3, copy_eng=nc.vector)

    nc.compile()
    return nc


def kernel(X, mask, W_Q, b_Q, W_K, b_K, W_V, b_V, W_O, b_O):
    global last_results
    import concourse.mybir as mybir
    from concourse.bass_utils import run_bass_kernel_spmd

    b16 = mybir.dt.np(mybir.dt.bfloat16)

    X = np.ascontiguousarray(X, dtype=np.float32)
    mask2 = np.asarray(mask).reshape(B, S) != 0
    counts = mask2.sum(axis=1)
    assert counts.min() >= 1
    SKV = max(P, int(math.ceil(counts.max() / P)) * P)

    XT = np.ascontiguousarray(X.transpose(0, 2, 1))          # (B, D, S)
    XKV = np.zeros((B, D_MODEL, SKV), dtype=np.float32)
    VF = np.zeros((B, SKV), dtype=np.float32)
    for b in range(B):
        idx = np.nonzero(mask2[b])[0]
        XKV[b, :, :len(idx)] = XT[b][:, idx]
        VF[b, :len(idx)] = 1.0

    nc = _build(SKV)

    in_maps = []
    for c in range(N_CORES):
        b, g = divmod(c, GROUPS)
        sl = slice(g * DH, (g + 1) * DH)
        in_maps.append({
            "xt": XT[b].astype(b16),
            "xkv": XKV[b].astype(b16),
            "wqt": np.ascontiguousarray(W_Q[sl, :].T).astype(b16),
            "wkt": np.ascontiguousarray(W_K[sl, :].T).astype(b16),
            "wvt": np.ascontiguousarray(W_V[sl, :].T).astype(b16),
            "wot": np.ascontiguousarray(W_O[:, sl].T).astype(b16),
            "bq": np.ascontiguousarray(b_Q[sl]),
            "bk": np.ascontiguousarray(b_K[sl]),
            "vf": VF[b].astype(b16),
        })

    trace_cores = None
    if os.environ.get("BASS_TRACE"):
        trace_cores = [int(x) for x in
                       os.environ.get("BASS_TRACE_CORES", "0").split(",")]
    res = run_bass_kernel_spmd(nc, in_maps, core_ids=list(range(N_CORES)),
                               trace_cores=trace_cores)
    last_results = res

    const = np.asarray(b_V, np.float64) @ np.asarray(W_O, np.float64).T \
        + np.asarray(b_O, np.float64)
    out = np.zeros((B, S, D_MODEL), dtype=np.float64)
    for c in range(N_CORES):
        b = c // GROUPS
        out[b] += res.results[c]["out"].astype(np.float64)
    out += const[None, None, :]
    return out.astype(np.float32)


# revision 8
# speedup vs baseline: 1.0581x; 1.0581x over previous
"""Multi-head attention (B=2, S=2048, D=1024, H=16) on 8 TRN2 NeuronCores.

Sharding: core c handles batch b = c//4 and head-group g = c%4 (4 heads,
d-slice of 256). Host compacts keys/values by the attention mask (exact:
masked keys contribute exp->0 in the fp32 reference), pads to a multiple
of 128; a valid-flag column excludes padding from numerator/denominator.

Per core (bf16 matmuls throughout, fp32 PSUM accumulation):
  Q^T = WqT.T @ X^T (+bq)          [256, 2048]
  K^T = WkT.T @ Xkv^T (+bk)        [256, SKV]
  V   = Xkv^T-chunks @ WvT         [SKV, 4h, 64+vf]
  per (head-pair, q-block): S^T = K_h Q_h^T for both heads concurrently
     (64x128 row-tiles T0/T8), P = exp(S^T/8) on Act in ONE [128,2kc,
     2head,512] activation per group (bf16 out)
  psO[65, 512] accumulates [V_h | vf].T @ P over key chunks:
     rows 0..63 = unnormalized O^T, row 64 = softmax denominator
  O^T = psO[0:64] * recip(den)  (den staged to SBUF - the custom-DVE
     reciprocal misreads PSUM sources on HW; GpSimd partition-broadcast)
  OUT_partial = O^T.T @ WoT        [2048, 1024], bf16 out

Scheduling: slots are (q-block, head-PAIR); the two heads share a KT/Q
column at partitions 0-63 / 64-127 so their score matmuls pack into the
PE as concurrent row-tiles (2x). Act is the near-critical engine in
steady state (~9.2us of exp per slot), so score groups are interleaved
with "filler" thunks (AV chunks of the previous pair, out-projection
nb-halves, next Q-projection) at 2-3 matmul granularity - the PE duty
cycle stays high enough that the HAM clock gate never re-throttles
(1.2 GHz cold penalties dominated the naive schedule). Throwaway junk
matmuls on a memset tile warm the PE during the DMA lead-in, pad the
DMA-paced K-projection, and bridge the normalization latency in the
tail. PSUM: one 4-bank scores slot (tag s4) ping-held per group via the
exp WAR, 4 single-bank b1 slots for everything else.

V/O biases fold into a host-side constant: A@(V+bv)Wo^T + bo =
A@V@Wo^T + (bv@Wo^T + bo). Partial outputs over head-groups are summed
on the host.
"""

import math
import os
from functools import lru_cache

import numpy as np

D_MODEL = 1024
NUM_HEADS = 16
D_K = 64


class _ActCopy:
    """tensor_copy shim routing through the Act engine's activation-Copy."""

    def __init__(self, nc):
        self.nc = nc

    def tensor_copy(self, out, in_):
        self.nc.scalar.copy(out, in_)


B = 2
S = 2048
N_CORES = 8
GROUPS = 4          # head-groups = cores per batch
DH = 256            # d-slice per core (4 heads x 64)
NH_LOC = 4          # heads per core
P = 128
CC = D_MODEL // P   # contraction chunks (8)

# results of the last hardware run (BassKernelResults), for test harnesses
last_results = None


@lru_cache(maxsize=2)
def _build(SKV: int):
    import concourse.mybir as mybir
    import concourse.tile as tile
    from concourse import bacc

    f32 = mybir.dt.float32
    bf16 = mybir.dt.bfloat16
    KC = SKV // P
    QB = S // 512                       # q blocks of 512
    NSLOT = QB * 2                      # (q block, head pair) slots
    kc_groups = [list(range(g, min(g + 2, KC))) for g in range(0, KC, 2)]
    kb512 = [(s0, min(512, SKV - s0)) for s0 in range(0, SKV, 512)]
    av_chunks = [range(0, 3), range(3, 6), range(6, KC)]

    nc = bacc.Bacc("TRN2", target_bir_lowering=False, debug=False,
                   num_devices=N_CORES)

    XT_d = nc.dram_tensor("xt", [D_MODEL, S], bf16, kind="ExternalInput").ap()
    XKV_d = nc.dram_tensor("xkv", [D_MODEL, SKV], bf16, kind="ExternalInput").ap()
    WQT_d = nc.dram_tensor("wqt", [D_MODEL, DH], bf16, kind="ExternalInput").ap()
    WKT_d = nc.dram_tensor("wkt", [D_MODEL, DH], bf16, kind="ExternalInput").ap()
    WVT_d = nc.dram_tensor("wvt", [D_MODEL, DH], bf16, kind="ExternalInput").ap()
    WOT_d = nc.dram_tensor("wot", [DH, D_MODEL], bf16, kind="ExternalInput").ap()
    bq_d = nc.dram_tensor("bq", [DH], f32, kind="ExternalInput").ap()
    bk_d = nc.dram_tensor("bk", [DH], f32, kind="ExternalInput").ap()
    vf_d = nc.dram_tensor("vf", [SKV], bf16, kind="ExternalInput").ap()
    OUT_d = nc.dram_tensor("out", [S, D_MODEL], bf16, kind="ExternalOutput").ap()

    with tile.TileContext(nc) as tc:
        with tc.tile_pool(name="res", bufs=1) as res, \
             tc.tile_pool(name="ps", bufs=2, space="PSUM") as ps, \
             tc.tile_pool(name="qtp", bufs=4) as qtp, \
             tc.tile_pool(name="ptp", bufs=2) as ptp, \
             tc.tile_pool(name="otp", bufs=4) as otp, \
             tc.tile_pool(name="nrm", bufs=6) as nrm:
            XKV_sb = res.tile([P, CC, SKV], bf16)
            WKT_sb = res.tile([P, CC, DH], bf16)
            WVT_sb = res.tile([P, CC, DH], bf16)
            WQT_sb = res.tile([P, CC, DH], bf16)
            XT_sb = res.tile([P, CC, S], bf16)
            WOT_sb = res.tile([P, 2, D_MODEL], bf16)
            bq_sb = res.tile([P, 2], f32)
            bk_sb = res.tile([P, 2], f32)
            KT_sb = res.tile([P, 2, SKV], bf16)
            V_sb = res.tile([P, KC, NH_LOC, 65], bf16)
            junk = res.tile([P, 640], bf16)

            # PE warm-up + filler state. The junk matmuls keep the HAM
            # clock gate at 2.4 GHz across DMA waits and drain latencies.
            # They draw fresh tiles from the s4 (scores) psum tag, which
            # is idle whenever junk is needed - b1 would deadlock the
            # in-order PE queue on a bank that only frees later.
            nc.vector.memset(junk[:], 0.0)
            jcount = [0]

            def emit_junk(n):
                pw = ps.tile([P, 2, 2, 512], f32, tag="s4", bufs=1,
                             name=f"warm{jcount[0]}")
                jcount[0] += 1
                for _ in range(n):
                    nc.tensor.matmul(pw[:, 0, 0, :], junk[:, 0:128],
                                     junk[:, 128:640], start=True, stop=True)

            emit_junk(8)

            # ---- input DMAs. SP (sync) queue: K-proj critical path in
            # arrival order, then biases, then the last X block. GpSimd
            # hardware queue: Q-proj inputs, V-proj inputs, early X blocks.
            nc.sync.dma_start(WKT_sb[:], WKT_d.rearrange("(c p) d -> p c d", p=P))
            for cq in range(0, CC, 2):
                nc.sync.dma_start(XKV_sb[:, cq:cq + 2, :],
                                  XKV_d.rearrange("(c p) k -> p c k", p=P)
                                      [:, cq:cq + 2, :])
            nc.sync.dma_start(bk_sb[:], bk_d.rearrange("(t p) -> p t", p=P))
            nc.sync.dma_start(bq_sb[:], bq_d.rearrange("(t p) -> p t", p=P))
            nc.sync.dma_start(
                XT_sb[:, :, 3 * 512:4 * 512],
                XT_d.rearrange("(c p) q -> p c q", p=P)[:, :, 3 * 512:4 * 512])

            nc.gpsimd.dma_start(WQT_sb[:], WQT_d.rearrange("(c p) d -> p c d", p=P))
            for qb in (0, 1, 2):
                nc.gpsimd.dma_start(
                    XT_sb[:, :, qb * 512:(qb + 1) * 512],
                    XT_d.rearrange("(c p) q -> p c q", p=P)
                        [:, :, qb * 512:(qb + 1) * 512])
            nc.gpsimd.dma_start(WVT_sb[:], WVT_d.rearrange("(c p) d -> p c d", p=P))
            for h in range(NH_LOC):
                nc.gpsimd.dma_start(V_sb[:, :, h, 64],
                                    vf_d.rearrange("(kc p) -> p kc", p=P))
            nc.gpsimd.dma_start(WOT_sb[:], WOT_d.rearrange("(t p) e -> p t e", p=P))

            def emit_kproj(t, pad=False):
                # i-outer over contraction chunks so K-proj consumes the
                # XKV DMA chunk-pairs as they land; junk-matmul padding
                # absorbs the arrival jitter without idling the PE.
                psks = []
                for j0, (k0, sz) in enumerate(kb512):
                    psks.append(ps.tile([P, 512], f32, tag="b1",
                                        name=f"psk{t}_{j0}"))
                for i in range(CC):
                    if pad and i in (2, 4, 6):
                        emit_junk(2)
                    for j0, (k0, sz) in enumerate(kb512):
                        nc.tensor.matmul(
                            psks[j0][:, 0:sz],
                            WKT_sb[:, i, t * P:(t + 1) * P],
                            XKV_sb[:, i, k0:k0 + sz],
                            start=(i == 0), stop=(i == CC - 1))
                for j0, (k0, sz) in enumerate(kb512):
                    nc.vector.tensor_scalar_add(
                        KT_sb[:, t, k0:k0 + sz], psks[j0][:, 0:sz],
                        bk_sb[:, t:t + 1])

            def emit_vproj(kc):
                psv = ps.tile([P, 512], f32, tag="b1", name=f"psv{kc}")
                for i in range(CC):
                    nc.tensor.matmul(
                        psv[:, 0:DH],
                        XKV_sb[:, i, kc * P:(kc + 1) * P],
                        WVT_sb[:, i, :],
                        start=(i == 0), stop=(i == CC - 1))
                nc.vector.tensor_copy(
                    V_sb[:, kc, :, 0:64],
                    psv[:, 0:DH].rearrange("p (h d) -> p h d", h=NH_LOC))

            qts = {}

            def qproj_thunks(qb):
                q0 = qb * 512

                def tthunk(t):
                    if t == 0:
                        qts[qb] = qtp.tile([P, 2, 512], bf16, tag="qt",
                                           name=f"qt{qb}")
                    qt = qts[qb]
                    psq = ps.tile([P, 512], f32, tag="b1", name=f"psq{qb}_{t}")
                    for i in range(CC):
                        nc.tensor.matmul(
                            psq[:],
                            WQT_sb[:, i, t * P:(t + 1) * P],
                            XT_sb[:, i, q0:q0 + 512],
                            start=(i == 0), stop=(i == CC - 1))
                    nc.vector.tensor_scalar_add(
                        qt[:, t, :], psq[:], bq_sb[:, t:t + 1])
                return [lambda t=t: tthunk(t) for t in range(2)]

            ptcs = {}

            def emit_scores(qb, hp, kcs):
                # both heads' score matmuls per kc, concurrent row-tiles;
                # one batched exp covers [kcs x 2 heads] from the 4-bank
                # s4 psum slot into the pair's P-tile.
                qt = qts[qb]
                if (qb, hp) in ptcs:
                    ptc = ptcs[(qb, hp)]
                else:
                    ptc = ptp.tile([P, 2, KC, 512], bf16, tag="pt",
                                   name=f"pt{qb}_{hp}")
                    ptcs[(qb, hp)] = ptc
                sg = ps.tile([P, 2, 2, 512], f32, tag="s4", bufs=1,
                             name=f"sg{qb}_{hp}_{kcs[0]}")
                for i, kc in enumerate(kcs):
                    for eo, po in enumerate((0, 64)):
                        nc.tensor.matmul(
                            sg[:, i, eo, :],
                            KT_sb[po:po + 64, hp, kc * P:(kc + 1) * P],
                            qt[po:po + 64, hp, :],
                            start=True, stop=True)
                n = len(kcs)
                nc.scalar.activation(
                    ptc.rearrange("p e k c -> p k e c")
                       [:, kcs[0]:kcs[0] + n, :, :],
                    sg[:, 0:n, :, :],
                    mybir.ActivationFunctionType.Exp, scale=0.125)

            ots = {}

            def av_thunks(qb, h):
                hp, eo = divmod(h, 2)
                t, po = hp, eo * 64
                st = {}

                def chunk(ci):
                    rng = av_chunks[ci]
                    if ci == 0:
                        st['pso'] = ps.tile([P, 512], f32, tag="b1",
                                            name=f"pso{qb}_{h}")
                    pso = st['pso']
                    ptc = ptcs[(qb, hp)]
                    for kc in rng:
                        nc.tensor.matmul(
                            pso[0:65, :],
                            V_sb[:, kc, h, :],
                            ptc[:, eo, kc, :],
                            start=(kc == 0), stop=(kc == KC - 1))
                    if ci == len(av_chunks) - 1:
                        # den -> SBUF (custom-DVE recip can't read PSUM)
                        den = nrm.tile([1, 512], f32, tag="den")
                        nc.vector.tensor_copy(den[:], pso[64:65, :])
                        rec = nrm.tile([1, 512], f32, tag="rec")
                        nc.vector.reciprocal_approx_fast(rec[:], den[:])
                        recb = nrm.tile([64, 512], f32, tag="recb")
                        nc.gpsimd.partition_broadcast(recb[:], rec[:],
                                                      channels=64)
                        if h == 0:
                            ots[qb] = otp.tile([P, 2, 512], bf16, tag="ot",
                                               name=f"ot{qb}")
                        nc.vector.tensor_mul(ots[qb][po:po + 64, t, :],
                                             pso[0:64, :], recb[:])
                return [lambda c=c: chunk(c) for c in range(len(av_chunks))]

            def oproj_thunks(qb, qc, copy_eng=None):
                q0 = qb * 512
                st = {}
                if copy_eng is None:
                    copy_eng = nc.vector

                def nbthunk(nb):
                    ot = ots[qb]
                    if nb == 0:
                        st['ob'] = nrm.tile([P, 2, 512], bf16, tag="ob",
                                            name=f"ob{qb}_{qc}")
                    ob = st['ob']
                    pso1 = ps.tile([P, 512], f32, tag="b1",
                                   name=f"op{qb}_{qc}_{nb}")
                    for t in range(2):
                        nc.tensor.matmul(
                            pso1[:],
                            ot[:, t, qc * P:(qc + 1) * P],
                            WOT_sb[:, t, nb * 512:(nb + 1) * 512],
                            start=(t == 0), stop=(t == 1))
                    copy_eng.tensor_copy(ob[:, nb, :], pso1[:])
                    if nb == 1:
                        nc.sync.dma_start(
                            OUT_d[q0 + qc * P:q0 + (qc + 1) * P, :], ob[:])
                return [lambda b=b: nbthunk(b) for b in range(2)]

            # ---- front: K/Q projections overlapped with the input DMA
            emit_kproj(0, pad=True)
            emit_junk(4)
            for th in qproj_thunks(0):
                th()

            # ---- software-pipelined (qb, head-pair) slots
            for s in range(NSLOT):
                qb, hp = divmod(s, 2)
                thunks = []
                if s == 0:
                    thunks += [lambda kc=kc: emit_vproj(kc)
                               for kc in range(0, 5)]
                    thunks += [lambda: emit_kproj(1)]
                    thunks += qproj_thunks(1)
                else:
                    pqb, php = divmod(s - 1, 2)
                    if s == 1:
                        # remaining V chunks BEFORE the AVs that read them
                        # (the in-order PE queue would deadlock otherwise)
                        thunks += [lambda kc=kc: emit_vproj(kc)
                                   for kc in range(5, KC)]
                    thunks += av_thunks(pqb, 2 * php)
                    thunks += av_thunks(pqb, 2 * php + 1)
                    if qb >= 1:
                        thunks += oproj_thunks(qb - 1, 2 * hp)
                        thunks += oproj_thunks(qb - 1, 2 * hp + 1)
                    if hp == 0 and qb + 1 < QB:
                        thunks += qproj_thunks(qb + 1)
                # interleave: score group, then a slice of the thunk list
                ngr = len(kc_groups)
                done = 0
                for g, kcs in enumerate(kc_groups):
                    emit_scores(qb, hp, kcs)
                    take = ((g + 1) * len(thunks)) // ngr
                    for th in thunks[done:take]:
                        th()
                    done = take

            # ---- tail: last pair's AV interleaved, junk to bridge the
            # normalization latency, then the final out-projections on
            # both copy engines.
            ta = av_thunks(QB - 1, 2)
            tb = av_thunks(QB - 1, 3)
            for x, y in zip(ta, tb):
                x()
                y()
            emit_junk(6)
            engs = [nc.vector, _ActCopy(nc), _ActCopy(nc), nc.vector]
            for qc in range(4):
                for th in oproj_thunks(QB - 1, qc, copy_eng=engs[qc]):
                    th()

    nc.compile()
    return nc


def kernel(X, mask, W_Q, b_Q, W_K, b_K, W_V, b_V, W_O, b_O):
    global last_results
    import concourse.mybir as mybir
    from concourse.bass_utils import run_bass_kernel_spmd

    b16 = mybir.dt.np(mybir.dt.bfloat16)

    X = np.ascontiguousarray(X, dtype=np.float32)
    mask2 = np.asarray(mask).reshape(B, S) != 0
    counts = mask2.sum(axis=1)
    assert counts.min() >= 1
    SKV = max(P, int(math.ceil(counts.max() / P)) * P)

    XT = np.ascontiguousarray(X.transpose(0, 2, 1))          # (B, D, S)
    XKV = np.zeros((B, D_MODEL, SKV), dtype=np.float32)
    VF = np.zeros((B, SKV), dtype=np.float32)
    for b in range(B):
        idx = np.nonzero(mask2[b])[0]
        XKV[b, :, :len(idx)] = XT[b][:, idx]
        VF[b, :len(idx)] = 1.0

    nc = _build(SKV)

    in_maps = []
    for c in range(N_CORES):
        b, g = divmod(c, GROUPS)
        sl = slice(g * DH, (g + 1) * DH)
        in_maps.append({
            "xt": XT[b].astype(b16),
            "xkv": XKV[b].astype(b16),
            "wqt": np.ascontiguousarray(W_Q[sl, :].T).astype(b16),
            "wkt": np.ascontiguousarray(W_K[sl, :].T).astype(b16),
            "wvt": np.ascontiguousarray(W_V[sl, :].T).astype(b16),
            "wot": np.ascontiguousarray(W_O[:, sl].T).astype(b16),
            "bq": np.ascontiguousarray(b_Q[sl]),
            "bk": np.ascontiguousarray(b_K[sl]),
            "vf": VF[b].astype(b16),
        })

    trace_cores = None
    if os.environ.get("BASS_TRACE"):
        trace_cores = [int(x) for x in
                       os.environ.get("BASS_TRACE_CORES", "0").split(",")]
    res = run_bass_kernel_spmd(nc, in_maps, core_ids=list(range(N_CORES)),
                               trace_cores=trace_cores)
    last_results = res

    const = np.asarray(b_V, np.float64) @ np.asarray(W_O, np.float64).T \
        + np.asarray(b_O, np.float64)
    out = np.zeros((B, S, D_MODEL), dtype=np.float64)
    for c in range(N_CORES):
        b = c // GROUPS
        out[b] += res.results[c]["out"].astype(np.float64)
    out += const[None, None, :]
    return out.astype(np.float32)


# revision 13
# speedup vs baseline: 1.0689x; 1.0102x over previous
"""Multi-head attention (B=2, S=2048, D=1024, H=16) on 8 TRN2 NeuronCores.

Sharding: core c handles batch b = c//4 and head-group g = c%4 (4 heads,
d-slice of 256). Host compacts keys/values by the attention mask (exact:
masked keys contribute exp->0 in the fp32 reference), pads to a multiple
of 128; a valid-flag column excludes padding from numerator/denominator.

Per core (bf16 matmuls throughout, fp32 PSUM accumulation):
  Q^T = WqT.T @ X^T (+bq)          [256, 2048]
  K^T = WkT.T @ Xkv^T (+bk)        [256, SKV]
  V   = Xkv^T-chunks @ WvT         [SKV, 4h, 64+vf]
  per (head-pair, q-block): S^T = K_h Q_h^T for both heads concurrently
     (64x128 row-tiles T0/T8), P = exp(S^T/8) on Act in ONE [128,2kc,
     2head,512] activation per group (bf16 out)
  psO[65, 512] accumulates [V_h | vf].T @ P over key chunks:
     rows 0..63 = unnormalized O^T, row 64 = softmax denominator
  O^T = psO[0:64] * recip(den)  (den staged to SBUF - the custom-DVE
     reciprocal misreads PSUM sources on HW; GpSimd partition-broadcast)
  OUT_partial = O^T.T @ WoT        [2048, 1024], bf16 out

Scheduling: slots are (q-block, head-PAIR); the two heads share a KT/Q
column at partitions 0-63 / 64-127 so their score matmuls pack into the
PE as concurrent row-tiles (2x). Act is the near-critical engine in
steady state (~9.2us of exp per slot), so score groups are interleaved
with "filler" thunks (AV chunks of the previous pair, out-projection
nb-halves, next Q-projection) at 2-3 matmul granularity - the PE duty
cycle stays high enough that the HAM clock gate never re-throttles
(1.2 GHz cold penalties dominated the naive schedule). Throwaway junk
matmuls on a memset tile warm the PE during the DMA lead-in, pad the
DMA-paced K-projection, and bridge the normalization latency in the
tail. PSUM: one 4-bank scores slot (tag s4) ping-held per group via the
exp WAR, 4 single-bank b1 slots for everything else.

V/O biases fold into a host-side constant: A@(V+bv)Wo^T + bo =
A@V@Wo^T + (bv@Wo^T + bo). Partial outputs over head-groups are summed
on the host.
"""

import math
import os
from functools import lru_cache

import numpy as np

D_MODEL = 1024
NUM_HEADS = 16
D_K = 64


class _ActCopy:
    """tensor_copy shim routing through the Act engine's activation-Copy."""

    def __init__(self, nc):
        self.nc = nc

    def tensor_copy(self, out, in_):
        self.nc.scalar.copy(out, in_)


B = 2
S = 2048
N_CORES = 8
GROUPS = 4          # head-groups = cores per batch
DH = 256            # d-slice per core (4 heads x 64)
NH_LOC = 4          # heads per core
P = 128
CC = D_MODEL // P   # contraction chunks (8)

# results of the last hardware run (BassKernelResults), for test harnesses
last_results = None


@lru_cache(maxsize=2)
def _build(SKV: int):
    import concourse.mybir as mybir
    import concourse.tile as tile
    from concourse import bacc

    f32 = mybir.dt.float32
    bf16 = mybir.dt.bfloat16
    KC = SKV // P
    QB = S // 512                       # q blocks of 512
    NSLOT = QB * 2                      # (q block, head pair) slots
    kc_groups = [list(range(g, min(g + 2, KC))) for g in range(0, KC, 2)]
    kb512 = [(s0, min(512, SKV - s0)) for s0 in range(0, SKV, 512)]
    av_chunks = [range(0, 3), range(3, 6), range(6, KC)]

    nc = bacc.Bacc("TRN2", target_bir_lowering=False, debug=False,
                   num_devices=N_CORES)

    XT_d = nc.dram_tensor("xt", [D_MODEL, S], bf16, kind="ExternalInput").ap()
    XKV_d = nc.dram_tensor("xkv", [D_MODEL, SKV], bf16, kind="ExternalInput").ap()
    WQT_d = nc.dram_tensor("wqt", [D_MODEL, DH], bf16, kind="ExternalInput").ap()
    WKT_d = nc.dram_tensor("wkt", [D_MODEL, DH], bf16, kind="ExternalInput").ap()
    WVT_d = nc.dram_tensor("wvt", [D_MODEL, DH], bf16, kind="ExternalInput").ap()
    WOT_d = nc.dram_tensor("wot", [DH, D_MODEL], bf16, kind="ExternalInput").ap()
    bq_d = nc.dram_tensor("bq", [DH], f32, kind="ExternalInput").ap()
    bk_d = nc.dram_tensor("bk", [DH], f32, kind="ExternalInput").ap()
    vf_d = nc.dram_tensor("vf", [SKV], bf16, kind="ExternalInput").ap()
    OUT_d = nc.dram_tensor("out", [S, D_MODEL], bf16, kind="ExternalOutput").ap()

    with tile.TileContext(nc) as tc:
        with tc.tile_pool(name="res", bufs=1) as res, \
             tc.tile_pool(name="ps", bufs=2, space="PSUM") as ps, \
             tc.tile_pool(name="qtp", bufs=4) as qtp, \
             tc.tile_pool(name="ptp", bufs=2) as ptp, \
             tc.tile_pool(name="otp", bufs=4) as otp, \
             tc.tile_pool(name="nrm", bufs=6) as nrm:
            XKV_sb = res.tile([P, CC, SKV], bf16)
            WKT_sb = res.tile([P, CC, DH], bf16)
            WVT_sb = res.tile([P, CC, DH], bf16)
            WQT_sb = res.tile([P, CC, DH], bf16)
            XT_sb = res.tile([P, CC, S], bf16)
            WOT_sb = res.tile([P, 2, D_MODEL], bf16)
            bq_sb = res.tile([P, 2], f32)
            bk_sb = res.tile([P, 2], f32)
            KT_sb = res.tile([P, 2, SKV], bf16)
            V_sb = res.tile([P, KC, NH_LOC, 65], bf16)
            junk = res.tile([P, 640], bf16)

            # PE warm-up + filler state. The junk matmuls keep the HAM
            # clock gate at 2.4 GHz across DMA waits, Act-bound slot
            # tails, and drain latencies. They get a DEDICATED psum bank
            # (tag jk) so a junk batch only ever waits on earlier junk,
            # never on real work draining - with a shared tag the fills
            # would stall on exactly the bubble they are meant to fill.
            nc.vector.memset(junk[:], 0.0)
            jcount = [0]

            def emit_junk(n):
                pw = ps.tile([P, 512], f32, tag="jk", bufs=1,
                             name=f"warm{jcount[0]}")
                jcount[0] += 1
                for _ in range(n):
                    nc.tensor.matmul(pw[:], junk[:, 0:128],
                                     junk[:, 128:640], start=True, stop=True)

            emit_junk(8)

            # ---- input DMAs. SP (sync) queue: K-proj critical path in
            # arrival order, then biases, then the last X block. GpSimd
            # hardware queue: Q-proj inputs, V-proj inputs, early X blocks.
            nc.sync.dma_start(WKT_sb[:], WKT_d.rearrange("(c p) d -> p c d", p=P))
            for cq in range(0, CC, 2):
                nc.sync.dma_start(XKV_sb[:, cq:cq + 2, :],
                                  XKV_d.rearrange("(c p) k -> p c k", p=P)
                                      [:, cq:cq + 2, :])
            nc.sync.dma_start(bk_sb[:], bk_d.rearrange("(t p) -> p t", p=P))
            nc.sync.dma_start(bq_sb[:], bq_d.rearrange("(t p) -> p t", p=P))
            nc.sync.dma_start(
                XT_sb[:, :, 3 * 512:4 * 512],
                XT_d.rearrange("(c p) q -> p c q", p=P)[:, :, 3 * 512:4 * 512])

            nc.gpsimd.dma_start(WQT_sb[:], WQT_d.rearrange("(c p) d -> p c d", p=P))
            for qb in (0, 1, 2):
                nc.gpsimd.dma_start(
                    XT_sb[:, :, qb * 512:(qb + 1) * 512],
                    XT_d.rearrange("(c p) q -> p c q", p=P)
                        [:, :, qb * 512:(qb + 1) * 512])
            nc.gpsimd.dma_start(WVT_sb[:], WVT_d.rearrange("(c p) d -> p c d", p=P))
            for h in range(NH_LOC):
                nc.gpsimd.dma_start(V_sb[:, :, h, 64],
                                    vf_d.rearrange("(kc p) -> p kc", p=P))
            nc.gpsimd.dma_start(WOT_sb[:], WOT_d.rearrange("(t p) e -> p t e", p=P))

            def emit_kproj(t, pad=False):
                # i-outer over contraction chunks so K-proj consumes the
                # XKV DMA chunk-pairs as they land; junk-matmul padding
                # absorbs the arrival jitter without idling the PE.
                psks = []
                for j0, (k0, sz) in enumerate(kb512):
                    psks.append(ps.tile([P, 512], f32, tag="b1",
                                        name=f"psk{t}_{j0}"))
                for i in range(CC):
                    if pad and i >= 1:
                        emit_junk(3)
                    for j0, (k0, sz) in enumerate(kb512):
                        nc.tensor.matmul(
                            psks[j0][:, 0:sz],
                            WKT_sb[:, i, t * P:(t + 1) * P],
                            XKV_sb[:, i, k0:k0 + sz],
                            start=(i == 0), stop=(i == CC - 1))
                for j0, (k0, sz) in enumerate(kb512):
                    nc.vector.tensor_scalar_add(
                        KT_sb[:, t, k0:k0 + sz], psks[j0][:, 0:sz],
                        bk_sb[:, t:t + 1])

            def emit_vproj(kc):
                psv = ps.tile([P, 512], f32, tag="b1", name=f"psv{kc}")
                for i in range(CC):
                    nc.tensor.matmul(
                        psv[:, 0:DH],
                        XKV_sb[:, i, kc * P:(kc + 1) * P],
                        WVT_sb[:, i, :],
                        start=(i == 0), stop=(i == CC - 1))
                nc.vector.tensor_copy(
                    V_sb[:, kc, :, 0:64],
                    psv[:, 0:DH].rearrange("p (h d) -> p h d", h=NH_LOC))

            qts = {}

            def qproj_thunks(qb):
                q0 = qb * 512

                def tthunk(t):
                    if t == 0:
                        qts[qb] = qtp.tile([P, 2, 512], bf16, tag="qt",
                                           name=f"qt{qb}")
                    qt = qts[qb]
                    psq = ps.tile([P, 512], f32, tag="b1", name=f"psq{qb}_{t}")
                    for i in range(CC):
                        nc.tensor.matmul(
                            psq[:],
                            WQT_sb[:, i, t * P:(t + 1) * P],
                            XT_sb[:, i, q0:q0 + 512],
                            start=(i == 0), stop=(i == CC - 1))
                    nc.vector.tensor_scalar_add(
                        qt[:, t, :], psq[:], bq_sb[:, t:t + 1])
                return [lambda t=t: tthunk(t) for t in range(2)]

            ptcs = {}

            def emit_scores(qb, hp, kcs):
                # both heads' score matmuls per kc, concurrent row-tiles;
                # one batched exp covers [kcs x 2 heads] from the 4-bank
                # s4 psum slot into the pair's P-tile.
                qt = qts[qb]
                if (qb, hp) in ptcs:
                    ptc = ptcs[(qb, hp)]
                else:
                    ptc = ptp.tile([P, 2, KC, 512], bf16, tag="pt",
                                   name=f"pt{qb}_{hp}")
                    ptcs[(qb, hp)] = ptc
                sg = ps.tile([P, 2, 2, 512], f32, tag="s4", bufs=1,
                             name=f"sg{qb}_{hp}_{kcs[0]}")
                for i, kc in enumerate(kcs):
                    for eo, po in enumerate((0, 64)):
                        nc.tensor.matmul(
                            sg[:, i, eo, :],
                            KT_sb[po:po + 64, hp, kc * P:(kc + 1) * P],
                            qt[po:po + 64, hp, :],
                            start=True, stop=True)
                n = len(kcs)
                nc.scalar.activation(
                    ptc.rearrange("p e k c -> p k e c")
                       [:, kcs[0]:kcs[0] + n, :, :],
                    sg[:, 0:n, :, :],
                    mybir.ActivationFunctionType.Exp, scale=0.125)

            ots = {}

            def av_thunks(qb, h):
                hp, eo = divmod(h, 2)
                t, po = hp, eo * 64
                st = {}

                def chunk(ci):
                    rng = av_chunks[ci]
                    if ci == 0:
                        st['pso'] = ps.tile([P, 512], f32, tag="b1",
                                            name=f"pso{qb}_{h}")
                    pso = st['pso']
                    ptc = ptcs[(qb, hp)]
                    for kc in rng:
                        nc.tensor.matmul(
                            pso[0:65, :],
                            V_sb[:, kc, h, :],
                            ptc[:, eo, kc, :],
                            start=(kc == 0), stop=(kc == KC - 1))
                    if ci == len(av_chunks) - 1:
                        # den -> SBUF (custom-DVE recip can't read PSUM)
                        den = nrm.tile([1, 512], f32, tag="den")
                        nc.vector.tensor_copy(den[:], pso[64:65, :])
                        rec = nrm.tile([1, 512], f32, tag="rec")
                        nc.vector.reciprocal_approx_fast(rec[:], den[:])
                        recb = nrm.tile([64, 512], f32, tag="recb")
                        nc.gpsimd.partition_broadcast(recb[:], rec[:],
                                                      channels=64)
                        if h == 0:
                            ots[qb] = otp.tile([P, 2, 512], bf16, tag="ot",
                                               name=f"ot{qb}")
                        nc.vector.tensor_mul(ots[qb][po:po + 64, t, :],
                                             pso[0:64, :], recb[:])
                return [lambda c=c: chunk(c) for c in range(len(av_chunks))]

            def oproj_thunks(qb, qc, copy_eng=None):
                q0 = qb * 512
                st = {}
                if copy_eng is None:
                    copy_eng = nc.vector

                def nbthunk(nb):
                    ot = ots[qb]
                    if nb == 0:
                        st['ob'] = nrm.tile([P, 2, 512], bf16, tag="ob",
                                            name=f"ob{qb}_{qc}")
                    ob = st['ob']
                    pso1 = ps.tile([P, 512], f32, tag="b1",
                                   name=f"op{qb}_{qc}_{nb}")
                    for t in range(2):
                        nc.tensor.matmul(
                            pso1[:],
                            ot[:, t, qc * P:(qc + 1) * P],
                            WOT_sb[:, t, nb * 512:(nb + 1) * 512],
                            start=(t == 0), stop=(t == 1))
                    copy_eng.tensor_copy(ob[:, nb, :], pso1[:])
                    if nb == 1:
                        nc.sync.dma_start(
                            OUT_d[q0 + qc * P:q0 + (qc + 1) * P, :], ob[:])
                return [lambda b=b: nbthunk(b) for b in range(2)]

            # ---- front: K/Q projections overlapped with the input DMA
            emit_kproj(0, pad=True)
            emit_junk(6)
            for th in qproj_thunks(0):
                th()

            # ---- software-pipelined (qb, head-pair) slots
            for s in range(NSLOT):
                qb, hp = divmod(s, 2)
                thunks = []
                if s == 0:
                    thunks += [lambda kc=kc: emit_vproj(kc)
                               for kc in range(0, 5)]
                    thunks += [lambda: emit_kproj(1)]
                    thunks += qproj_thunks(1)
                else:
                    pqb, php = divmod(s - 1, 2)
                    if s == 1:
                        # remaining V chunks BEFORE the AVs that read them
                        # (the in-order PE queue would deadlock otherwise)
                        thunks += [lambda kc=kc: emit_vproj(kc)
                                   for kc in range(5, KC)]
                    thunks += av_thunks(pqb, 2 * php)
                    thunks += av_thunks(pqb, 2 * php + 1)
                    if qb >= 1:
                        # 1/3 split balances PE work against the fixed
                        # ~9us of exp per slot (hp0 also carries qproj)
                        if hp == 0:
                            thunks += oproj_thunks(qb - 1, 0)
                        else:
                            for qc in (1, 2, 3):
                                thunks += oproj_thunks(qb - 1, qc)
                    if hp == 0 and qb + 1 < QB:
                        thunks += qproj_thunks(qb + 1)
                # interleave: score group, then a slice of the thunk list
                ngr = len(kc_groups)
                done = 0
                for g, kcs in enumerate(kc_groups):
                    emit_scores(qb, hp, kcs)
                    take = ((g + 1) * len(thunks)) // ngr
                    for th in thunks[done:take]:
                        th()
                    done = take
                if s >= 2:
                    emit_junk(8 if s == 2 * QB - 2 else 3)

            # ---- tail: last pair's AV interleaved, junk to bridge the
            # normalization latency, then the final out-projections on
            # both copy engines.
            ta = av_thunks(QB - 1, 2)
            tb = av_thunks(QB - 1, 3)
            for x, y in zip(ta, tb):
                x()
                y()
            emit_junk(10)
            engs = [nc.vector, _ActCopy(nc), _ActCopy(nc), nc.vector]
            for qc in range(4):
                for th in oproj_thunks(QB - 1, qc, copy_eng=engs[qc]):
                    th()
                if qc == 1:
                    emit_junk(4)

    nc.compile()
    return nc


def kernel(X, mask, W_Q, b_Q, W_K, b_K, W_V, b_V, W_O, b_O):
    global last_results
    import concourse.mybir as mybir
    from concourse.bass_utils import run_bass_kernel_spmd

    b16 = mybir.dt.np(mybir.dt.bfloat16)

    X = np.ascontiguousarray(X, dtype=np.float32)
    mask2 = np.asarray(mask).reshape(B, S) != 0
    counts = mask2.sum(axis=1)
    assert counts.min() >= 1
    SKV = max(P, int(math.ceil(counts.max() / P)) * P)

    XT = np.ascontiguousarray(X.transpose(0, 2, 1))          # (B, D, S)
    XKV = np.zeros((B, D_MODEL, SKV), dtype=np.float32)
    VF = np.zeros((B, SKV), dtype=np.float32)
    for b in range(B):
        idx = np.nonzero(mask2[b])[0]
        XKV[b, :, :len(idx)] = XT[b][:, idx]
        VF[b, :len(idx)] = 1.0

    nc = _build(SKV)

    in_maps = []
    for c in range(N_CORES):
        b, g = divmod(c, GROUPS)
        sl = slice(g * DH, (g + 1) * DH)
        in_maps.append({
            "xt": XT[b].astype(b16),
            "xkv": XKV[b].astype(b16),
            "wqt": np.ascontiguousarray(W_Q[sl, :].T).astype(b16),
            "wkt": np.ascontiguousarray(W_K[sl, :].T).astype(b16),
            "wvt": np.ascontiguousarray(W_V[sl, :].T).astype(b16),
            "wot": np.ascontiguousarray(W_O[:, sl].T).astype(b16),
            "bq": np.ascontiguousarray(b_Q[sl]),
            "bk": np.ascontiguousarray(b_K[sl]),
            "vf": VF[b].astype(b16),
        })

    trace_cores = None
    if os.environ.get("BASS_TRACE"):
        trace_cores = [int(x) for x in
                       os.environ.get("BASS_TRACE_CORES", "0").split(",")]
    res = run_bass_kernel_spmd(nc, in_maps, core_ids=list(range(N_CORES)),
                               trace_cores=trace_cores)
    last_results = res

    const = np.asarray(b_V, np.float64) @ np.asarray(W_O, np.float64).T \
        + np.asarray(b_O, np.float64)
    out = np.zeros((B, S, D_MODEL), dtype=np.float64)
    for c in range(N_CORES):
        b = c // GROUPS
        out[b] += res.results[c]["out"].astype(np.float64)
    out += const[None, None, :]
    return out.astype(np.float32)


# revision 16
# speedup vs baseline: 1.0772x; 1.0078x over previous
"""Multi-head attention (B=2, S=2048, D=1024, H=16) on 8 TRN2 NeuronCores.

Sharding: core c handles batch b = c//4 and head-group g = c%4 (4 heads,
d-slice of 256). Host compacts keys/values by the attention mask (exact:
masked keys contribute exp->0 in the fp32 reference), pads to a multiple
of 128; a valid-flag column excludes padding from numerator/denominator.

Per core (bf16 matmuls throughout, fp32 PSUM accumulation):
  Q^T = WqT.T @ X^T (+bq)          [256, 2048]
  K^T = WkT.T @ Xkv^T (+bk)        [256, SKV]
  V   = Xkv^T-chunks @ WvT         [SKV, 4h, 64+vf]
  per (head-pair, q-block): S^T = K_h Q_h^T for both heads concurrently
     (64x128 row-tiles T0/T8), P = exp(S^T/8) on Act in ONE [128,2kc,
     2head,512] activation per group (bf16 out)
  psO[65, 512] accumulates [V_h | vf].T @ P over key chunks:
     rows 0..63 = unnormalized O^T, row 64 = softmax denominator
  O^T = psO[0:64] * recip(den)  (den staged to SBUF - the custom-DVE
     reciprocal misreads PSUM sources on HW; GpSimd partition-broadcast)
  OUT_partial = O^T.T @ WoT        [2048, 1024], bf16 out

Scheduling: slots are (q-block, head-PAIR); the two heads share a KT/Q
column at partitions 0-63 / 64-127 so their score matmuls pack into the
PE as concurrent row-tiles (2x). Act is the near-critical engine in
steady state (~9.2us of exp per slot), so score groups are interleaved
with "filler" thunks (AV chunks of the previous pair, out-projection
nb-halves, next Q-projection) at 2-3 matmul granularity - the PE duty
cycle stays high enough that the HAM clock gate never re-throttles
(1.2 GHz cold penalties dominated the naive schedule). Throwaway junk
matmuls on a memset tile warm the PE during the DMA lead-in, pad the
DMA-paced K-projection, and bridge the normalization latency in the
tail. PSUM: one 4-bank scores slot (tag s4) ping-held per group via the
exp WAR, 4 single-bank b1 slots for everything else.

V/O biases fold into a host-side constant: A@(V+bv)Wo^T + bo =
A@V@Wo^T + (bv@Wo^T + bo). Partial outputs over head-groups are summed
on the host.
"""

import math
import os
from functools import lru_cache

import numpy as np

D_MODEL = 1024
NUM_HEADS = 16
D_K = 64


class _ActCopy:
    """tensor_copy shim routing through the Act engine's activation-Copy."""

    def __init__(self, nc):
        self.nc = nc

    def tensor_copy(self, out, in_):
        self.nc.scalar.copy(out, in_)


B = 2
S = 2048
N_CORES = 8
GROUPS = 4          # head-groups = cores per batch
DH = 256            # d-slice per core (4 heads x 64)
NH_LOC = 4          # heads per core
P = 128
CC = D_MODEL // P   # contraction chunks (8)

# results of the last hardware run (BassKernelResults), for test harnesses
last_results = None


@lru_cache(maxsize=2)
def _build(SKV: int):
    import concourse.mybir as mybir
    import concourse.tile as tile
    from concourse import bacc

    f32 = mybir.dt.float32
    bf16 = mybir.dt.bfloat16
    KC = SKV // P
    QB = S // 512                       # q blocks of 512
    NSLOT = QB * 2                      # (q block, head pair) slots
    kc_groups = [list(range(g, min(g + 2, KC))) for g in range(0, KC, 2)]
    kb512 = [(s0, min(512, SKV - s0)) for s0 in range(0, SKV, 512)]
    av_chunks = [range(0, 3), range(3, 6), range(6, KC)]

    nc = bacc.Bacc("TRN2", target_bir_lowering=False, debug=False,
                   num_devices=N_CORES)

    XT_d = nc.dram_tensor("xt", [D_MODEL, S], bf16, kind="ExternalInput").ap()
    XKV_d = nc.dram_tensor("xkv", [D_MODEL, SKV], bf16, kind="ExternalInput").ap()
    WQT_d = nc.dram_tensor("wqt", [D_MODEL, DH], bf16, kind="ExternalInput").ap()
    WKT_d = nc.dram_tensor("wkt", [D_MODEL, DH], bf16, kind="ExternalInput").ap()
    WVT_d = nc.dram_tensor("wvt", [D_MODEL, DH], bf16, kind="ExternalInput").ap()
    WOT_d = nc.dram_tensor("wot", [DH, D_MODEL], bf16, kind="ExternalInput").ap()
    bq_d = nc.dram_tensor("bq", [DH], f32, kind="ExternalInput").ap()
    bk_d = nc.dram_tensor("bk", [DH], f32, kind="ExternalInput").ap()
    vf_d = nc.dram_tensor("vf", [SKV], bf16, kind="ExternalInput").ap()
    OUT_d = nc.dram_tensor("out", [S, D_MODEL], bf16, kind="ExternalOutput").ap()

    with tile.TileContext(nc) as tc:
        with tc.tile_pool(name="res", bufs=1) as res, \
             tc.tile_pool(name="ps", bufs=2, space="PSUM") as ps, \
             tc.tile_pool(name="qtp", bufs=4) as qtp, \
             tc.tile_pool(name="ptp", bufs=2) as ptp, \
             tc.tile_pool(name="otp", bufs=4) as otp, \
             tc.tile_pool(name="nrm", bufs=6) as nrm:
            XKV_sb = res.tile([P, CC, SKV], bf16)
            WKT_sb = res.tile([P, CC, DH], bf16)
            WVT_sb = res.tile([P, CC, DH], bf16)
            WQT_sb = res.tile([P, CC, DH], bf16)
            XT_sb = res.tile([P, CC, S], bf16)
            WOT_sb = res.tile([P, 2, D_MODEL], bf16)
            bq_sb = res.tile([P, 2], f32)
            bk_sb = res.tile([P, 2], f32)
            KT_sb = res.tile([P, 2, SKV], bf16)
            V_sb = res.tile([P, KC, NH_LOC, 65], bf16)
            junk = res.tile([P, 640], bf16)

            # PE warm-up + filler state. The junk matmuls keep the HAM
            # clock gate at 2.4 GHz across DMA waits, Act-bound slot
            # tails, and drain latencies. They get a DEDICATED psum bank
            # (tag jk) so a junk batch only ever waits on earlier junk,
            # never on real work draining - with a shared tag the fills
            # would stall on exactly the bubble they are meant to fill.
            nc.vector.memset(junk[:], 0.0)
            jcount = [0]

            def emit_junk(n):
                pw = ps.tile([P, 512], f32, tag="jk", bufs=1,
                             name=f"warm{jcount[0]}")
                jcount[0] += 1
                for _ in range(n):
                    nc.tensor.matmul(pw[:], junk[:, 0:128],
                                     junk[:, 128:640], start=True, stop=True)

            emit_junk(8)

            # ---- input DMAs. SP (sync) queue: K-proj critical path in
            # arrival order, then biases, then the last X block. GpSimd
            # hardware queue: Q-proj inputs, V-proj inputs, early X blocks.
            nc.sync.dma_start(WKT_sb[:], WKT_d.rearrange("(c p) d -> p c d", p=P))
            for cq in range(0, CC, 2):
                nc.sync.dma_start(XKV_sb[:, cq:cq + 2, :],
                                  XKV_d.rearrange("(c p) k -> p c k", p=P)
                                      [:, cq:cq + 2, :])
            nc.sync.dma_start(bk_sb[:], bk_d.rearrange("(t p) -> p t", p=P))
            nc.sync.dma_start(bq_sb[:], bq_d.rearrange("(t p) -> p t", p=P))
            nc.sync.dma_start(
                XT_sb[:, :, 3 * 512:4 * 512],
                XT_d.rearrange("(c p) q -> p c q", p=P)[:, :, 3 * 512:4 * 512])

            nc.gpsimd.dma_start(WQT_sb[:], WQT_d.rearrange("(c p) d -> p c d", p=P))
            for qb in (0, 1, 2):
                nc.gpsimd.dma_start(
                    XT_sb[:, :, qb * 512:(qb + 1) * 512],
                    XT_d.rearrange("(c p) q -> p c q", p=P)
                        [:, :, qb * 512:(qb + 1) * 512])
            nc.gpsimd.dma_start(WVT_sb[:], WVT_d.rearrange("(c p) d -> p c d", p=P))
            for h in range(NH_LOC):
                nc.gpsimd.dma_start(V_sb[:, :, h, 64],
                                    vf_d.rearrange("(kc p) -> p kc", p=P))
            nc.gpsimd.dma_start(WOT_sb[:], WOT_d.rearrange("(t p) e -> p t e", p=P))

            def emit_kproj(t, pad=False):
                # i-outer over contraction chunks so K-proj consumes the
                # XKV DMA chunk-pairs as they land; junk-matmul padding
                # absorbs the arrival jitter without idling the PE.
                psks = []
                for j0, (k0, sz) in enumerate(kb512):
                    psks.append(ps.tile([P, 512], f32, tag="b1",
                                        name=f"psk{t}_{j0}"))
                for i in range(CC):
                    if pad and i >= 1:
                        emit_junk(3)
                    for j0, (k0, sz) in enumerate(kb512):
                        nc.tensor.matmul(
                            psks[j0][:, 0:sz],
                            WKT_sb[:, i, t * P:(t + 1) * P],
                            XKV_sb[:, i, k0:k0 + sz],
                            start=(i == 0), stop=(i == CC - 1))
                for j0, (k0, sz) in enumerate(kb512):
                    nc.vector.tensor_scalar_add(
                        KT_sb[:, t, k0:k0 + sz], psks[j0][:, 0:sz],
                        bk_sb[:, t:t + 1])

            def emit_vproj(kc):
                psv = ps.tile([P, 512], f32, tag="b1", name=f"psv{kc}")
                for i in range(CC):
                    nc.tensor.matmul(
                        psv[:, 0:DH],
                        XKV_sb[:, i, kc * P:(kc + 1) * P],
                        WVT_sb[:, i, :],
                        start=(i == 0), stop=(i == CC - 1))
                nc.vector.tensor_copy(
                    V_sb[:, kc, :, 0:64],
                    psv[:, 0:DH].rearrange("p (h d) -> p h d", h=NH_LOC))

            qts = {}

            def qproj_thunks(qb):
                q0 = qb * 512

                def tthunk(t):
                    if t == 0:
                        qts[qb] = qtp.tile([P, 2, 512], bf16, tag="qt",
                                           name=f"qt{qb}")
                    qt = qts[qb]
                    psq = ps.tile([P, 512], f32, tag="b1", name=f"psq{qb}_{t}")
                    for i in range(CC):
                        nc.tensor.matmul(
                            psq[:],
                            WQT_sb[:, i, t * P:(t + 1) * P],
                            XT_sb[:, i, q0:q0 + 512],
                            start=(i == 0), stop=(i == CC - 1))
                    nc.vector.tensor_scalar_add(
                        qt[:, t, :], psq[:], bq_sb[:, t:t + 1])
                return [lambda t=t: tthunk(t) for t in range(2)]

            ptcs = {}

            def emit_scores(qb, hp, kcs):
                # both heads' score matmuls per kc, concurrent row-tiles;
                # one batched exp covers [kcs x 2 heads] from the 4-bank
                # s4 psum slot into the pair's P-tile.
                qt = qts[qb]
                if (qb, hp) in ptcs:
                    ptc = ptcs[(qb, hp)]
                else:
                    ptc = ptp.tile([P, 2, KC, 512], bf16, tag="pt",
                                   name=f"pt{qb}_{hp}")
                    ptcs[(qb, hp)] = ptc
                sg = ps.tile([P, 2, 2, 512], f32, tag="s4", bufs=1,
                             name=f"sg{qb}_{hp}_{kcs[0]}")
                for i, kc in enumerate(kcs):
                    for eo, po in enumerate((0, 64)):
                        nc.tensor.matmul(
                            sg[:, i, eo, :],
                            KT_sb[po:po + 64, hp, kc * P:(kc + 1) * P],
                            qt[po:po + 64, hp, :],
                            start=True, stop=True)
                n = len(kcs)
                nc.scalar.activation(
                    ptc.rearrange("p e k c -> p k e c")
                       [:, kcs[0]:kcs[0] + n, :, :],
                    sg[:, 0:n, :, :],
                    mybir.ActivationFunctionType.Exp, scale=0.125)

            ots = {}

            def av_thunks(qb, h):
                hp, eo = divmod(h, 2)
                t, po = hp, eo * 64
                st = {}

                def chunk(ci):
                    rng = av_chunks[ci]
                    if ci == 0:
                        st['pso'] = ps.tile([P, 512], f32, tag="b1",
                                            name=f"pso{qb}_{h}")
                    pso = st['pso']
                    ptc = ptcs[(qb, hp)]
                    for kc in rng:
                        nc.tensor.matmul(
                            pso[0:65, :],
                            V_sb[:, kc, h, :],
                            ptc[:, eo, kc, :],
                            start=(kc == 0), stop=(kc == KC - 1))
                    if ci == len(av_chunks) - 1:
                        # den -> SBUF (custom-DVE recip can't read PSUM)
                        den = nrm.tile([1, 512], f32, tag="den")
                        nc.vector.tensor_copy(den[:], pso[64:65, :])
                        rec = nrm.tile([1, 512], f32, tag="rec")
                        nc.vector.reciprocal_approx_fast(rec[:], den[:])
                        recb = nrm.tile([64, 512], f32, tag="recb")
                        nc.gpsimd.partition_broadcast(recb[:], rec[:],
                                                      channels=64)
                        if h == 0:
                            ots[qb] = otp.tile([P, 2, 512], bf16, tag="ot",
                                               name=f"ot{qb}")
                        nc.vector.tensor_mul(ots[qb][po:po + 64, t, :],
                                             pso[0:64, :], recb[:])
                return [lambda c=c: chunk(c) for c in range(len(av_chunks))]

            def oproj_thunks(qb, qc, copy_eng=None):
                q0 = qb * 512
                st = {}
                if copy_eng is None:
                    copy_eng = nc.vector

                def nbthunk(nb):
                    ot = ots[qb]
                    if nb == 0:
                        st['ob'] = nrm.tile([P, 2, 512], bf16, tag="ob",
                                            name=f"ob{qb}_{qc}")
                    ob = st['ob']
                    pso1 = ps.tile([P, 512], f32, tag="b1",
                                   name=f"op{qb}_{qc}_{nb}")
                    for t in range(2):
                        nc.tensor.matmul(
                            pso1[:],
                            ot[:, t, qc * P:(qc + 1) * P],
                            WOT_sb[:, t, nb * 512:(nb + 1) * 512],
                            start=(t == 0), stop=(t == 1))
                    copy_eng.tensor_copy(ob[:, nb, :], pso1[:])
                    if nb == 1:
                        nc.sync.dma_start(
                            OUT_d[q0 + qc * P:q0 + (qc + 1) * P, :], ob[:])
                return [lambda b=b: nbthunk(b) for b in range(2)]

            # ---- front: K/Q projections overlapped with the input DMA
            emit_kproj(0, pad=True)
            emit_junk(16)
            for th in qproj_thunks(0):
                th()

            # ---- software-pipelined (qb, head-pair) slots
            for s in range(NSLOT):
                qb, hp = divmod(s, 2)
                thunks = []
                if s == 0:
                    thunks += [lambda kc=kc: emit_vproj(kc)
                               for kc in range(0, 5)]
                    thunks += [lambda: emit_kproj(1)]
                    thunks += qproj_thunks(1)
                else:
                    pqb, php = divmod(s - 1, 2)
                    if s == 1:
                        # remaining V chunks BEFORE the AVs that read them
                        # (the in-order PE queue would deadlock otherwise)
                        thunks += [lambda kc=kc: emit_vproj(kc)
                                   for kc in range(5, KC)]
                    thunks += av_thunks(pqb, 2 * php)
                    thunks += av_thunks(pqb, 2 * php + 1)
                    if qb >= 1:
                        # 1/3 split balances PE work against the fixed
                        # ~9us of exp per slot (hp0 also carries qproj)
                        if hp == 0:
                            thunks += oproj_thunks(qb - 1, 0)
                        else:
                            for qc in (1, 2, 3):
                                thunks += oproj_thunks(qb - 1, qc)
                    if hp == 0 and qb + 1 < QB:
                        thunks += qproj_thunks(qb + 1)
                # interleave: score group, then a slice of the thunk list
                ngr = len(kc_groups)
                done = 0
                for g, kcs in enumerate(kc_groups):
                    emit_scores(qb, hp, kcs)
                    take = ((g + 1) * len(thunks)) // ngr
                    for th in thunks[done:take]:
                        th()
                    done = take
                if s >= 2:
                    emit_junk(6 if s >= 2 * QB - 3 else 3)

            # ---- tail: last pair's AV interleaved, junk to bridge the
            # normalization latency, then the final out-projections on
            # both copy engines.
            ta = av_thunks(QB - 1, 2)
            tb = av_thunks(QB - 1, 3)
            for x, y in zip(ta, tb):
                x()
                y()
            emit_junk(18)
            engs = [nc.vector, _ActCopy(nc), _ActCopy(nc), nc.vector]
            for qc in range(4):
                for th in oproj_thunks(QB - 1, qc, copy_eng=engs[qc]):
                    th()
                if qc == 1:
                    emit_junk(4)

    nc.compile()
    return nc


def kernel(X, mask, W_Q, b_Q, W_K, b_K, W_V, b_V, W_O, b_O):
    global last_results
    import concourse.mybir as mybir
    from concourse.bass_utils import run_bass_kernel_spmd

    b16 = mybir.dt.np(mybir.dt.bfloat16)

    X = np.ascontiguousarray(X, dtype=np.float32)
    mask2 = np.asarray(mask).reshape(B, S) != 0
    counts = mask2.sum(axis=1)
    assert counts.min() >= 1
    SKV = max(P, int(math.ceil(counts.max() / P)) * P)

    XT = np.ascontiguousarray(X.transpose(0, 2, 1))          # (B, D, S)
    XKV = np.zeros((B, D_MODEL, SKV), dtype=np.float32)
    VF = np.zeros((B, SKV), dtype=np.float32)
    for b in range(B):
        idx = np.nonzero(mask2[b])[0]
        XKV[b, :, :len(idx)] = XT[b][:, idx]
        VF[b, :len(idx)] = 1.0

    nc = _build(SKV)

    in_maps = []
    for c in range(N_CORES):
        b, g = divmod(c, GROUPS)
        sl = slice(g * DH, (g + 1) * DH)
        in_maps.append({
            "xt": XT[b].astype(b16),
            "xkv": XKV[b].astype(b16),
            "wqt": np.ascontiguousarray(W_Q[sl, :].T).astype(b16),
            "wkt": np.ascontiguousarray(W_K[sl, :].T).astype(b16),
            "wvt": np.ascontiguousarray(W_V[sl, :].T).astype(b16),
            "wot": np.ascontiguousarray(W_O[:, sl].T).astype(b16),
            "bq": np.ascontiguousarray(b_Q[sl]),
            "bk": np.ascontiguousarray(b_K[sl]),
            "vf": VF[b].astype(b16),
        })

    trace_cores = None
    if os.environ.get("BASS_TRACE"):
        trace_cores = [int(x) for x in
                       os.environ.get("BASS_TRACE_CORES", "0").split(",")]
    res = run_bass_kernel_spmd(nc, in_maps, core_ids=list(range(N_CORES)),
                               trace_cores=trace_cores)
    last_results = res

    const = np.asarray(b_V, np.float64) @ np.asarray(W_O, np.float64).T \
        + np.asarray(b_O, np.float64)
    out = np.zeros((B, S, D_MODEL), dtype=np.float64)
    for c in range(N_CORES):
        b = c // GROUPS
        out[b] += res.results[c]["out"].astype(np.float64)
    out += const[None, None, :]
    return out.astype(np.float32)
